# revision 1
# baseline (speedup 1.0000x reference)
"""Trainium2 Bass kernel for multi-head causal self-attention.

Problem: B=4, S=2048, D=768, H=12, DH=64 (fp32).
  Q = x @ W_Q + b_Q; K, V likewise
  scores = QK^T / sqrt(DH), causal mask, softmax
  out = (probs @ V) @ W_O + b_O

Sharding over 8 cores: core c -> batch b = c//2, head-half hh = c%2
(6 heads each). Fully local compute, no collectives; the two partial
outputs per batch (head-halves) are summed on the host during unshard.

Device layout is "transposed" everywhere (sequence on the free dim):
  xT   [D, S]       (host pre-transposes)
  QT,KT[384, S]     hk on partitions (3 chunks of 128 = 2 heads)
  V    [S, 453]     natural layout, per-head 65 cols (64 V + 1 ones col
                    so that P^T@[V|1] also accumulates softmax sums in the
                    same PSUM accumulation; PV uses a 128-wide lhsT window)
  S^T  [keys, q]    scores transposed -> softmax sum over keys is done
                    by the ones column in the PV matmul
  Z^T  [384, S]     normalized attention output
  outT [D, S]       host transposes back

Softmax skips the max-subtraction (scores are ~N(0, 0.3); exp is safe)
which is mathematically identical to the reference softmax.
"""

import numpy as np

import concourse.mybir as mybir
import concourse.tile as tile
from concourse import bacc, bass_utils

F32 = mybir.dt.float32
F32R = mybir.dt.float32r
BF16 = mybir.dt.bfloat16

B, S, D, H, DH = 4, 2048, 768, 12, 64
HL = 6                # heads per core
HK = HL * DH          # 384
NPAIR = HL // 2       # 3 chunks of 2 heads (128 partitions each)
P = 128
NDT = D // P          # 6 d-tiles
NST = S // P          # 16 key tiles
QH = 1024             # q half width (PSUM-friendly)
VW = DH + 1           # 65 = V cols + ones col
SCALE = 1.0 / 8.0     # 1/sqrt(DH)

# compute dtype for tensor-engine inputs ("f32r" = fp32 reduced / 1-pass)
COMPUTE = "bf16"


def _mm_dt(ap):
    if COMPUTE == "f32r" and ap.dtype == F32:
        return ap.bitcast(F32R)
    return ap


def _sb_dt():
    return BF16 if COMPUTE == "bf16" else F32


def _np_in(a):
    a = np.ascontiguousarray(a, dtype=np.float32)
    if COMPUTE == "bf16":
        import ml_dtypes
        return a.astype(ml_dtypes.bfloat16)
    return a


def _regions(o):
    """Split [o, QH) into per-PSUM-bank matmul regions (<=512 wide)."""
    if o < 512:
        return [(o, 512), (512, QH)]
    return [(o, QH)]


def build_nc():
    nc = bacc.Bacc("TRN2", target_bir_lowering=False, debug=False, num_devices=8)
    CD = _sb_dt()

    xT = nc.dram_tensor("xT", [D, S], CD, kind="ExternalInput").ap()
    wq = nc.dram_tensor("wq", [D, HK], CD, kind="ExternalInput").ap()
    wk = nc.dram_tensor("wk", [D, HK], CD, kind="ExternalInput").ap()
    wv = nc.dram_tensor("wv", [D, HK], CD, kind="ExternalInput").ap()
    wo = nc.dram_tensor("wo", [HK, D], CD, kind="ExternalInput").ap()
    bq = nc.dram_tensor("bq", [HK], F32, kind="ExternalInput").ap()
    bk = nc.dram_tensor("bk", [HK], F32, kind="ExternalInput").ap()
    bv = nc.dram_tensor("bv", [HK], CD, kind="ExternalInput").ap()
    bo = nc.dram_tensor("bo", [D], F32, kind="ExternalInput").ap()
    # additive causal mask for the diagonal 128x128 block, [key, q] layout,
    # -1e4 where q < key; applied as iden.T @ mska inside the S accumulation
    mska = nc.dram_tensor("mska", [P, P], CD, kind="ExternalInput").ap()
    iden = nc.dram_tensor("iden", [P, P], CD, kind="ExternalInput").ap()
    out = nc.dram_tensor("out", [D, S], F32, kind="ExternalOutput").ap()

    EXP = mybir.ActivationFunctionType.Exp

    with tile.TileContext(nc) as tc:
        with (
            tc.tile_pool(name="big", bufs=1) as big,
            tc.tile_pool(name="wts", bufs=1) as wts,
            tc.tile_pool(name="vpool", bufs=1) as vpool,
            tc.tile_pool(name="pp", bufs=6) as pp,
            tc.tile_pool(name="small", bufs=1) as small,
            tc.tile_pool(name="rcp", bufs=3) as rcp,
            tc.tile_pool(name="ot", bufs=6) as otp,
            tc.tile_pool(name="ps", bufs=4, space="PSUM") as ps,
        ):
            # ---- constants / biases -------------------------------------
            # Q/K/O biases ride the per-partition bias of the ACT copy out of
            # PSUM; the V bias (free-dim) is a rank-1 outer(ones, bias) K=1
            # matmul appended to the projection's accumulation group
            mska_sb = small.tile([P, P], CD, tag="mska")
            nc.gpsimd.dma_start(out=mska_sb, in_=mska)
            iden_sb = small.tile([P, P], CD, tag="iden")
            nc.gpsimd.dma_start(out=iden_sb, in_=iden)
            # touch Exp once at t=0 so the ACT table load (~2.7us) overlaps
            # the input DMA phase instead of stalling the first real exp
            warm_sb = small.tile([1, 8], F32, tag="warm")
            nc.vector.memset(warm_sb, 1.0)
            nc.scalar.activation(warm_sb, warm_sb, EXP)
            bq_sb = small.tile([P, NPAIR], F32, tag="bq")
            nc.gpsimd.dma_start(out=bq_sb, in_=bq.rearrange("(c p) -> p c", p=P))
            bk_sb = small.tile([P, NPAIR], F32, tag="bk")
            nc.gpsimd.dma_start(out=bk_sb, in_=bk.rearrange("(c p) -> p c", p=P))
            bv_sb = small.tile([1, HK], CD, tag="bv")
            nc.gpsimd.dma_start(out=bv_sb, in_=bv.rearrange("(o k) -> o k", o=1))
            bo_sb = small.tile([P, NDT], F32, tag="bo")
            nc.gpsimd.dma_start(out=bo_sb, in_=bo.rearrange("(c p) -> p c", p=P))
            ones_row = small.tile([1, 512], CD, tag="ones")
            nc.vector.memset(ones_row, 1.0)

            # ---- x ------------------------------------------------------
            xt = []
            for dt in range(NDT):
                t = big.tile([P, S], CD, tag=f"xt{dt}")
                eng = nc.scalar if dt < 3 else nc.gpsimd
                for ch in range(4):
                    eng.dma_start(
                        out=t[:, ch * 512:(ch + 1) * 512],
                        in_=xT[dt * P:(dt + 1) * P, ch * 512:(ch + 1) * 512],
                    )
                xt.append(t)

            # ---- weights ------------------------------------------------
            # issue in consumption order: all wq (first projection) first
            wq_sb = []
            wk_sb = []
            wv_sb = []
            for dram, lst, nm in ((wq, wq_sb, "wq"), (wk, wk_sb, "wk"), (wv, wv_sb, "wv")):
                for dt in range(NDT):
                    t = wts.tile([P, HK], CD, tag=f"{nm}{dt}", name=f"{nm}{dt}")
                    nc.sync.dma_start(out=t, in_=dram[dt * P:(dt + 1) * P, :])
                    lst.append(t)
            wo_sb = []
            for c in range(NPAIR):
                t = wts.tile([P, D], CD, tag=f"wo{c}")
                nc.sync.dma_start(out=t, in_=wo[c * P:(c + 1) * P, :])
                wo_sb.append(t)

            # ---- Q/K projections (transposed layout) --------------------
            QT = [big.tile([P, S], CD, tag=f"qt{c}", name=f"qt{c}") for c in range(NPAIR)]
            KT = [big.tile([P, S], CD, tag=f"kt{c}", name=f"kt{c}") for c in range(NPAIR)]
            for w_sb, b_sb, dst in ((wq_sb, bq_sb, QT), (wk_sb, bk_sb, KT)):
                for c in range(NPAIR):
                    for qc in range(S // 512):
                        pt = ps.tile([P, QH], F32, tag="ps")
                        for dt in range(NDT):
                            nc.tensor.matmul(
                                pt[:, 0:512],
                                lhsT=_mm_dt(w_sb[dt][:, c * P:(c + 1) * P]),
                                rhs=_mm_dt(xt[dt][:, qc * 512:(qc + 1) * 512]),
                                start=(dt == 0),
                                stop=(dt == NDT - 1),
                            )
                        nc.scalar.add(
                            dst[c][:, qc * 512:(qc + 1) * 512],
                            pt[:, 0:512],
                            b_sb[:, c:c + 1],
                        )

            # ---- V projection (natural layout, ones col per head) -------
            Vt = []
            for st in range(NST):
                pt = ps.tile([P, QH], F32, tag="ps")
                for dt in range(NDT):
                    nc.tensor.matmul(
                        pt[:, 0:HK],
                        lhsT=_mm_dt(xt[dt][:, st * P:(st + 1) * P]),
                        rhs=_mm_dt(wv_sb[dt]),
                        start=(dt == 0),
                        stop=False,
                    )
                nc.tensor.matmul(
                    pt[:, 0:HK],
                    lhsT=_mm_dt(ones_row[:, 0:P]),
                    rhs=_mm_dt(bv_sb),
                    start=False,
                    stop=True,
                )
                vt = vpool.tile([P, HL * VW + 63], CD, tag=f"v{st}")
                nc.gpsimd.memset(vt[:, HL * VW:], 0.0)
                vv = vt[:, 0:HL * VW].rearrange("p (h c) -> p h c", c=VW)
                nc.scalar.copy(
                    vv[:, :, 0:DH],
                    pt[:, 0:HK].rearrange("p (h c) -> p h c", c=DH),
                )
                nc.gpsimd.memset(vv[:, :, DH:VW], 1.0)
                Vt.append(vt)

            # ---- attention ----------------------------------------------
            # processed in head pairs: head a lives on partitions 0:64 of the
            # QT/KT chunk, head b on 64:128 -> their S^T matmuls (K=64) target
            # disjoint PE row groups and run concurrently (row tile_position)
            ZT = [big.tile([P, S], CD, tag=f"xt{c}", name=f"zt{c}") for c in range(NPAIR)]
            for pr in range(NPAIR):
                for qh in range(S // QH):
                    q0 = qh * QH
                    nkt = min(8 * (qh + 1), NST)
                    Oab = [
                        ps.tile([P, QH], F32, tag="ps", name=f"o{hh}")
                        for hh in range(2)
                    ]
                    a_last = 8 * qh + 3
                    b_last = nkt - 1
                    for kt in range(nkt):
                        o = max(0, P * kt - q0)
                        Sab = [
                            ps.tile([P, QH], F32, tag="ps", name=f"s{hh}")
                            for hh in range(2)
                        ]
                        diag = P * kt >= q0
                        for (a, b) in _regions(o):
                            for hh in range(2):
                                lo = hh * DH
                                nc.tensor.matmul(
                                    Sab[hh][:, a:b],
                                    lhsT=_mm_dt(KT[pr][lo:lo + DH, kt * P:(kt + 1) * P]),
                                    rhs=_mm_dt(QT[pr][lo:lo + DH, q0 + a:q0 + b]),
                                    start=True,
                                    stop=not (diag and a == o),
                                )
                        if diag:
                            # additive -1e4 on the strictly-lower (q < key)
                            # triangle of the diagonal block, via PE
                            for hh in range(2):
                                nc.tensor.matmul(
                                    Sab[hh][:, o:o + P],
                                    lhsT=_mm_dt(iden_sb),
                                    rhs=_mm_dt(mska_sb),
                                    start=False,
                                    stop=True,
                                )
                        Pts = []
                        for hh in range(2):
                            Pt = pp.tile([P, QH], CD, tag="p", name=f"p{hh}")
                            nc.scalar.activation(
                                Pt[:, o:QH], Sab[hh][:, o:QH], EXP, scale=SCALE
                            )
                            Pts.append(Pt)
                        for (a, b) in _regions(o):
                            last = a_last if b <= 512 else b_last
                            for hh in range(2):
                                h65 = (2 * pr + hh) * VW
                                nc.tensor.matmul(
                                    Oab[hh][:, a:b],
                                    lhsT=_mm_dt(Vt[kt][:, h65:h65 + P]),
                                    rhs=_mm_dt(Pts[hh][:, a:b]),
                                    start=(kt == 0),
                                    stop=(kt == last),
                                )
                    # normalize: ZT = O[:DH] * (1/sums) broadcast over partitions.
                    # O is copied to SBUF first so its PSUM slot frees
                    # immediately and the PE can start the next head pair.
                    for hh in range(2):
                        lo = hh * DH
                        # one copy of numerator+sums -> O's PSUM slot frees
                        # immediately; everything below reads SBUF
                        ocp = rcp.tile([VW, QH], F32, tag="ocp")
                        nc.vector.tensor_copy(ocp, Oab[hh][0:VW, :])
                        # reciprocal runs at ~6.4ns per FREE element (per
                        # lane), so spread the sums row over 32 partitions
                        # with the DVE 32x32 stream transpose first: the
                        # valid values land in column 0 of each 32-block
                        smt = rcp.tile([32, QH], F32, tag="smt")
                        nc.vector.tensor_copy(smt[0:1, :], ocp[DH:VW, :])
                        tt = rcp.tile([32, QH], F32, tag="tt")
                        nc.vector.transpose(tt, smt)
                        tv = tt.rearrange("p (j c) -> p j c", c=32)[:, :, 0:1]
                        nc.vector.reciprocal(tv, tv)
                        rc32 = rcp.tile([32, QH], F32, tag="rc32")
                        nc.vector.transpose(rc32, tt)
                        for (a, b) in ((0, 512), (512, QH)):
                            Rb = rcp.tile([DH, 512], F32, tag="rb")
                            nc.gpsimd.partition_broadcast(Rb, rc32[0:1, a:b])
                            nc.vector.tensor_mul(
                                ZT[pr][lo:lo + DH, q0 + a:q0 + b],
                                ocp[0:DH, a:b], Rb
                            )

            # ---- output projection --------------------------------------
            # qc-major: qc 0/1 depend only on the first q-half of ZT, so the
            # PE can fill the last head pair's reciprocal/normalize tail
            for qc in range(S // 512):
                for dt in range(NDT):
                    pt = ps.tile([P, QH], F32, tag="ps")
                    for c in range(NPAIR):
                        nc.tensor.matmul(
                            pt[:, 0:512],
                            lhsT=_mm_dt(wo_sb[c][:, dt * P:(dt + 1) * P]),
                            rhs=_mm_dt(ZT[c][:, qc * 512:(qc + 1) * 512]),
                            start=(c == 0),
                            stop=(c == NPAIR - 1),
                        )
                    osb = otp.tile([P, 512], F32, tag="ot")
                    nc.scalar.add(osb, pt[:, 0:512], bo_sb[:, dt:dt + 1])
                    nc.sync.dma_start(
                        out=out[dt * P:(dt + 1) * P, qc * 512:(qc + 1) * 512],
                        in_=osb,
                    )
    nc.compile()
    return nc


_NC_CACHE = {}


def _get_nc():
    if "nc" not in _NC_CACHE:
        _NC_CACHE["nc"] = build_nc()
    return _NC_CACHE["nc"]


def make_in_maps(x, W_Q, W_K, W_V, W_O, b_Q, b_K, b_V, b_O):
    mask_add = np.tril(np.full((P, P), -1e4, np.float32), k=-1)
    identity = np.eye(P, dtype=np.float32)
    in_maps = []
    for c in range(8):
        b, hh = divmod(c, 2)
        hs = slice(HL * hh, HL * hh + HL)
        in_maps.append({
            "xT": _np_in(x[b].T),
            "wq": _np_in(W_Q[hs].transpose(1, 0, 2).reshape(D, HK)),
            "wk": _np_in(W_K[hs].transpose(1, 0, 2).reshape(D, HK)),
            "wv": _np_in(W_V[hs].transpose(1, 0, 2).reshape(D, HK)),
            "wo": _np_in(W_O[hs].reshape(HK, D)),
            "bq": np.ascontiguousarray(b_Q[hs].reshape(HK), np.float32),
            "bk": np.ascontiguousarray(b_K[hs].reshape(HK), np.float32),
            "bv": _np_in(b_V[hs].reshape(HK)),
            "bo": np.ascontiguousarray(b_O if hh == 0 else np.zeros(D), np.float32),
            "mska": _np_in(mask_add),
            "iden": _np_in(identity),
        })
    return in_maps


def run(inputs, trace=False):
    nc = _get_nc()
    in_maps = make_in_maps(**inputs)
    res = bass_utils.run_bass_kernel_spmd(
        nc, in_maps, core_ids=list(range(8)), trace=trace,
        **({"trace_cores": [0]} if trace else {}),
    )
    outs = [r["out"] for r in res.results]
    full = np.empty((B, S, D), np.float32)
    for b in range(B):
        full[b] = (outs[2 * b] + outs[2 * b + 1]).T
    return full, res


def kernel(**inputs):
    full, _ = run(inputs)
    return full

